# revision 13
# baseline (speedup 1.0000x reference)
"""Trainium2 Bass kernel for MoGNN forward (global mean-pool + linear).

The model's conv outputs are discarded; the result depends only on x:
    pooled[g] = mean over nodes n with batch[n] == g of x[n]   # [1024, 512]
    out = pooled @ W.T + b                                     # [1024, 7]

batch ids are sorted, so nodes of each graph are contiguous. We shard by
GRAPHS: core k owns graphs [128k, 128k+128) and exactly the contiguous row
range of x belonging to them. No collectives.

x is shipped as fp8e4m3 with HOST-SIDE ERROR FEEDBACK: within each graph the
quantization error of node n is carried into node n+1 before quantizing, so
the per-graph SUM sees only the final carry (~one quantization step) instead
of sqrt(count) accumulated noise. Measured end-to-end relative error vs the
fp32 reference ~2.6e-3 (gate 2e-2) at a quarter of the fp32 HBM traffic.

BLOCK-INTERLEAVED DoubleRow: each core's graphs split into two blocks of 64;
both blocks are padded to the same tile count and their tiles are processed
in lockstep. One fp8 DoubleRow matmul takes block-0 tile t and block-1 tile
t as its two 128-deep k-tiles (256-deep contraction, 2 tiles per 512-cycle
instruction, full 128-column array as the ISA requires). Its stationary
reads a 128-col strip per tile: block-0 one-hot lives in cols [0,64) with
cols [64,128) permanently zero, block-1 in cols [64,128) with [0,64) zero.
The zero halves are memset once at startup by the otherwise-idle GpSimd
engine and survive pool recycling, so the DVE one-hot compare is only 64
wide per tile -- half the work of a full-width one-hot:
oh[n, w] = (iota[w] == local_id[n]) via one tensor_tensor(is_equal) per DMA
chunk per block with step-0 broadcast APs. PSUM is zeroed once up front;
matmuls run with start=False and skip_group_check.

Epilogue: PSUM -> SBUF with a per-graph 1/count scale (mean pool, fp16),
4x PE transpose to feat-major, then 4 fp16 matmuls with pooled.T stationary
and the W chunk moving (N=7, fp32 PSUM), bias added via a partition-
replicated fp32 tile; each core writes out[128, 7] and the host concatenates
to [1024, 7].
"""

import numpy as np

try:
    import ml_dtypes
except ImportError:  # pragma: no cover
    ml_dtypes = None

NCORES = 8
G = 1024            # total graphs
GPC = G // NCORES   # graphs per core = 128
F = 512             # feature dim
P = 128             # partition / node-tile size
CHUNK = 8           # block-tile indices per DMA chunk (2x8 tiles = 1 MB fp8)
NB = 2              # graph blocks per core (DoubleRow k-tiles)
WIN = GPC // NB     # 64 graphs per block

_compiled_cache = {}
_lut_cache = {}


def _fp8_luts():
    """uint16 (fp16 bits) -> (fp8e4m3 byte code, decoded fp32 value)."""
    if "c" not in _lut_cache:
        with np.errstate(invalid="ignore"):
            f8 = np.arange(65536, dtype=np.uint16).view(np.float16).astype(
                ml_dtypes.float8_e4m3
            )
        _lut_cache["c"] = f8.view(np.uint8)
        _lut_cache["d"] = f8.astype(np.float32)
    return _lut_cache["c"], _lut_cache["d"]


def _chunk_plan(btiles):
    """Chunk boundaries over BLOCK-tile indices: small leading chunks so the
    PE pipeline starts early, CHUNK steady state, small tail taper."""
    head = [2, 6]
    tail = [2]
    main_end = max(btiles - sum(tail), 0)
    chunks = []
    t0 = 0
    for ramp in head:
        if t0 < main_end:
            clen = min(ramp, main_end - t0)
            chunks.append((t0, clen))
            t0 += clen
    while t0 < main_end:
        clen = min(CHUNK, main_end - t0)
        chunks.append((t0, clen))
        t0 += clen
    while t0 < btiles:
        clen = min(CHUNK, btiles - t0)
        chunks.append((t0, clen))
        t0 += clen
    assert sum(c for _, c in chunks) == btiles
    return chunks


def _build(btiles):
    """Build + compile the per-core Bass kernel: NB blocks of `btiles` node
    tiles each, processed in lockstep by block-interleaved DoubleRow."""
    from concourse import bacc, tile, mybir

    f32 = mybir.dt.float32
    f16 = mybir.dt.float16
    f8 = mybir.dt.float8e4
    eq = mybir.AluOpType.is_equal
    mult = mybir.AluOpType.mult
    add = mybir.AluOpType.add
    DR = mybir.MatmulPerfMode.DoubleRow

    ntiles = NB * btiles
    nrows = ntiles * P
    chunks = _chunk_plan(btiles)
    OHB = 4  # oh pool buffers (their zero halves are memset once each)

    nc = bacc.Bacc(
        "TRN2",
        target_bir_lowering=False,
        debug=False,
        num_devices=NCORES,
    )

    # x laid out chunk-contiguous: chunk (c0, clen) holds, per partition p,
    # block[j][t][m] = x_block_j[(c0+t)*128+p, m] for j in {0,1} -- one
    # contiguous 2*clen*512-byte run per partition per chunk.
    x_d = nc.dram_tensor("xs", [nrows * F], f8, kind="ExternalInput")
    # constants: cpa = [bl | iota] feeds the one-hot build and goes FIRST on
    # the scalar ring; cpb = [ident | wtr] and cp32 = [b_replicated | icnt]
    # are epilogue-only and follow behind on the same ring. bl column j*btiles+t
    # is block j's tile t (values: full-core local graph ids, -1 pad).
    cpa_d = nc.dram_tensor("cpa", [P, ntiles + GPC], f16, kind="ExternalInput")
    cpb_d = nc.dram_tensor("cpb", [P, P + 28], f16, kind="ExternalInput")
    cp32_d = nc.dram_tensor("cp32", [P, 8], f32, kind="ExternalInput")
    out_d = nc.dram_tensor("out", [GPC, 7], f32, kind="ExternalOutput")

    with tile.TileContext(nc) as tc:
        with (
            tc.tile_pool(name="const", bufs=1) as constp,
            tc.tile_pool(name="xin", bufs=8) as xp,
            tc.tile_pool(name="oh", bufs=OHB) as ohp,
            tc.tile_pool(name="acc", bufs=1, space="PSUM") as accp,
            tc.tile_pool(name="tps", bufs=2, space="PSUM") as tpsp,
            tc.tile_pool(name="sb", bufs=2) as sbp,
        ):
            cpa_t = constp.tile([P, ntiles + GPC], f16)
            nc.scalar.dma_start(cpa_t[:], cpa_d.ap())
            cp32_t = constp.tile([P, 8], f32)
            nc.scalar.dma_start(cp32_t[:], cp32_d.ap())
            cpb_t = constp.tile([P, P + 28], f16)
            nc.scalar.dma_start(cpb_t[:], cpb_d.ap())
            bl_t = cpa_t[:, 0:ntiles]
            iota_t = cpa_t[:, ntiles : ntiles + GPC]
            ident_t = cpb_t[:, 0:P]
            wtr_t = cpb_t[:, P : P + 28]
            brep_t = cp32_t[:, 0:7]
            icnt_t = cp32_t[:, 7:8]

            acc = accp.tile([GPC, F], f32)
            nc.vector.memset(acc[:], 0.0)
            x_flat = x_d.ap()

            # pre-allocate the oh ring and zero the permanently-zero halves
            # on GpSimd (idle during the stream); the DVE one-hot writes only
            # the 64 live columns of each strip afterwards.
            oh_bufs = []
            for i in range(OHB):
                oh = ohp.tile([P, NB, CHUNK, GPC], f8, tag="oh")
                nc.gpsimd.memset(oh[:, 0, :, WIN:GPC], 0.0)
                nc.gpsimd.memset(oh[:, 1, :, 0:WIN], 0.0)
                oh_bufs.append(oh)

            iota_rep = iota_t.rearrange("p (a g) -> p a g", a=1)
            for ci, (c0, clen) in enumerate(chunks):
                xt = xp.tile([P, NB, CHUNK, F], f8, tag="xt")
                chunk_ap = x_flat[
                    c0 * NB * P * F : (c0 + clen) * NB * P * F
                ].rearrange("(p j t m) -> p j t m", p=P, j=NB, m=F)
                nc.sync.dma_start(xt[:, :, :clen, :], chunk_ap)
                oh = oh_bufs[ci % OHB]
                # narrow one-hot per block: block 0 compares iota[0:64],
                # block 1 compares iota[64:128]; values are full-core local
                # ids so each block's compare lands in its own column half.
                for j in range(NB):
                    wsl = slice(j * WIN, (j + 1) * WIN)
                    nc.vector.tensor_tensor(
                        oh[:, j, :clen, wsl],
                        iota_rep[:, :, wsl].broadcast_to([P, clen, WIN]),
                        bl_t[:, j * btiles + c0 : j * btiles + c0 + clen]
                        .rearrange("p (n a) -> p n a", a=1)
                        .broadcast_to([P, clen, WIN]),
                        op=eq,
                    )
                for n in range(clen):
                    nc.tensor.matmul(
                        acc[:],
                        oh[:, :, n, :],
                        xt[:, :, n, :],
                        start=False,
                        stop=False,
                        perf_mode=DR,
                        skip_group_check=True,
                    )

            # pooled = acc * (1/count[g]) cast to fp16, sliced so the (fp16,
            # full-rate) transposes pipeline behind the scale copies; then the
            # classifier with pooled.T as stationary (moving is W [128, 7], N=7)
            pooled = sbp.tile([GPC, F], f16)
            ptall = sbp.tile([P, 4, P], f16)
            for j in range(4):
                sl = slice(j * P, (j + 1) * P)
                nc.vector.tensor_scalar(
                    pooled[:, sl], acc[:, sl], icnt_t, None, op0=mult
                )
                tp = tpsp.tile([P, P], f16)
                nc.tensor.transpose(tp[:], pooled[:, sl], ident_t)
                nc.vector.tensor_copy(ptall[:, j, :], tp[:])

            out_ps = tpsp.tile([GPC, 7], f32, tag="outp")
            for j in range(4):
                nc.tensor.matmul(
                    out_ps[:],
                    ptall[:, j, :],
                    wtr_t[:, j * 7 : (j + 1) * 7],
                    start=(j == 0),
                    stop=(j == 3),
                )

            out_sb = sbp.tile([GPC, 7], f32)
            nc.vector.tensor_tensor(out_sb[:], out_ps[:], brep_t, op=add)
            nc.sync.dma_start(out_d.ap(), out_sb[:])

    nc.compile()
    return nc


def _get_compiled(btiles):
    if btiles not in _compiled_cache:
        _compiled_cache[btiles] = _build(btiles)
    return _compiled_cache[btiles]


def _ef_quantize(x, batch, counts):
    """fp8e4m3 codes of x with per-graph error feedback along the node axis.

    batch must be sorted. Returns uint8 codes, shape (N, F)."""
    lc, ld = _fp8_luts()
    Gn = counts.shape[0]
    starts = np.zeros(Gn, np.int64)
    starts[1:] = np.cumsum(counts)[:-1]
    codes = np.zeros(x.shape, np.uint8)
    err = np.zeros((Gn, x.shape[1]), np.float32)
    maxc = int(counts.max())
    for j in range(maxc):
        act = counts > j
        rows = (starts + j)[act]
        t = x[rows] + err[act]
        bits = t.astype(np.float16).view(np.uint16)
        codes[rows] = lc[bits]
        err[act] = t - ld[bits]
    return codes


def _prep_in_maps(codes, batch, W, b, btiles, block_bounds, inv_counts):
    ntiles = NB * btiles
    rows_per_block = btiles * P
    chunk_plan = _chunk_plan(btiles)
    iota = np.tile(np.arange(GPC, dtype=np.float16)[None, :], (P, 1))
    # wtr[p, c*7+j] = W.T[c*128+p, j]
    wtr = np.ascontiguousarray(
        W.T.reshape(4, P, 7).transpose(1, 0, 2).reshape(P, 28)
    ).astype(np.float16)
    cpb = np.empty((P, P + 28), dtype=np.float16)
    cpb[:, 0:P] = np.eye(P, dtype=np.float16)
    cpb[:, P:] = wtr
    cp32_base = np.zeros((P, 8), dtype=np.float32)
    cp32_base[:, 0:7] = b.astype(np.float32)[None, :]

    in_maps = []
    for k in range(NCORES):
        # per-block row data, padded to rows_per_block
        xsb = np.zeros((NB, rows_per_block, F), dtype=np.uint8)
        blvb = np.full((NB, rows_per_block), -1.0, dtype=np.float16)
        for j in range(NB):
            lo = int(block_bounds[k * NB + j])
            hi = int(block_bounds[k * NB + j + 1])
            n = hi - lo
            xsb[j, :n] = codes[lo:hi]
            blvb[j, :n] = (batch[lo:hi] - GPC * k).astype(np.float16)
        # chunk layout [p, j, t, m], contiguous per chunk
        xsb = xsb.reshape(NB, btiles, P, F)
        parts = [
            np.ascontiguousarray(
                xsb[:, c0 : c0 + clen].transpose(2, 0, 1, 3)
            ).reshape(-1)
            for c0, clen in chunk_plan
        ]
        xs = np.concatenate(parts).view(ml_dtypes.float8_e4m3)
        cpa = np.empty((P, ntiles + GPC), dtype=np.float16)
        # bl column j*btiles + t = block j, tile t
        cpa[:, 0:ntiles] = (
            blvb.reshape(NB * btiles, P).T
        )
        cpa[:, ntiles : ntiles + GPC] = iota
        cp32 = cp32_base.copy()
        cp32[:, 7] = inv_counts[GPC * k : GPC * (k + 1)]
        in_maps.append({"xs": xs, "cpa": cpa, "cpb": cpb, "cp32": cp32})
    return in_maps


_last_result = None  # test harness can read exec_time_ns / trace from here


def kernel(x, edge_index, edge_attr, batch_size, W, b):
    from concourse import bass_utils

    global _last_result

    x = np.asarray(x, dtype=np.float32)
    batch = np.asarray(batch_size).astype(np.int64)
    W = np.asarray(W, dtype=np.float32)
    b = np.asarray(b, dtype=np.float32)

    if batch.size > 1 and np.any(np.diff(batch) < 0):
        # contiguous-shard logic needs sorted ids; reordering nodes does not
        # change per-graph sums
        order = np.argsort(batch, kind="stable")
        batch = batch[order]
        x = x[order]

    counts = np.bincount(batch, minlength=G)
    inv_counts = (1.0 / np.maximum(counts, 1)).astype(np.float32)

    block_bounds = np.searchsorted(batch, np.arange(0, G + 1, WIN))
    max_block_rows = int(np.diff(block_bounds).max())
    btiles = max(-(-max_block_rows // P), 1)

    codes = _ef_quantize(x, batch, counts)

    nc = _get_compiled(btiles)
    in_maps = _prep_in_maps(
        codes, batch, W, b, btiles, block_bounds, inv_counts
    )

    res = bass_utils.run_bass_kernel_spmd(
        nc, in_maps, core_ids=list(range(NCORES))
    )
    _last_result = res

    # each core returns out [128, 7] for its graphs; assemble [1024, 7]
    out = np.concatenate(
        [np.asarray(res.results[k]["out"]) for k in range(NCORES)], axis=0
    )
    return np.ascontiguousarray(out.astype(np.float32))
